# revision 22
# baseline (speedup 1.0000x reference)
"""DeepseekV3 MLA attention forward on 8 Trainium2 NeuronCores.

Sharding: core c -> batch c//4, head group c%4 (4 of 16 heads).

v3: all five GEMM stages run as fp8(e4m3) DoubleRow matmuls with hi/lo
error compensation (3-product scheme: hi*whi + hi*wlo + lo*whi), which the
cost model prices at 0.5 cycles/row -- 2x the bf16 MAC rate for the same
accuracy class as bf16. Weights and the hidden input are split on the host;
device-computed intermediates (latents, q, kn, attn) are split with one Act
copy (hi) + one DVE scalar_tensor_tensor (lo) from the f32 PSUM product.

Attention PV is restructured v-stationary: out[vd, q] accumulates over key
tiles in a single PSUM bank per (head, chunk), so the per-q-tile reciprocal/
scale/transpose chains of v2 disappear. Softmax denominators come from
1-column ones matmuls per (q-tile, key-tile), reciprocals are transposed and
partition-broadcast, and one DVE multiply produces the normalized attn tile
(directly in the [vd, token] layout stage D consumes -- no transposes).

Score tiles are stored unshifted (diagonal tiles keep their causal zero
region) so PV and denominator matmuls are uniform full-width accumulations.

Stage A remains token-sharded across the 4 cores of each batch group with
chunk 0 replicated; per-chunk AllGathers carry the latents as hi/lo fp8
planes (same bytes as the old bf16 payload) plus the bf16 rope key.
"""

import math

import numpy as np
import ml_dtypes

import concourse.bass as bass
import concourse.tile as tile
import concourse.mybir as mybir
from concourse import bacc
from concourse import masks as bmasks
from concourse.bass_utils import run_bass_kernel_spmd

BF16 = mybir.dt.bfloat16
F32 = mybir.dt.float32
F8 = mybir.dt.float8e4
E4 = ml_dtypes.float8_e4m3
AF = mybir.ActivationFunctionType
DR = mybir.MatmulPerfMode.DoubleRow
MUL = mybir.AluOpType.mult
SUB = mybir.AluOpType.subtract

# ---- model config (hardcoded to match the problem spec) ----
HIDDEN = 2048
N_HEADS = 16
Q_LORA = 1536
KV_LORA = 512
NOPE = 128
ROPE = 64
VHD = 128
QHD = NOPE + ROPE  # 192
BASE = 10000.0
SCALE = 40.0
ORIG_MAX = 4096
BETA_FAST = 32
BETA_SLOW = 1
EPS = 1e-6
B = 2
S = 2048

N_CORES = 8
HL = 4           # heads per core
P = 128
KH = HIDDEN // P     # 16
KQ = Q_LORA // P     # 12
KKV = KV_LORA // P   # 4
NPL = 34             # gather planes: 16 ft x (hi,lo) fp8 + krot bf16 (2B)
CW = 512             # max chunk width / psum tile width
CHUNKS = [(0, 512), (512, 512), (1024, 512), (1536, 384), (1920, 128)]
NGATH = 3            # chunks 1..3 gathered; chunks 0 and 4 replicated
_sl_bases = []
_acc = 512
for _, w in CHUNKS[1:1 + NGATH]:
    _sl_bases.append((_acc, w // 4))
    _acc += w // 4
OWN_T = 512 + sum(w for _, w in CHUNKS[1:1 + NGATH]) // 4 + CHUNKS[4][1]  # 992

_m = 0.1 * math.log(SCALE) + 1.0
SOFT_SCALE = (QHD ** -0.5) * _m * _m

# fp8 scales
S_H = 16.0          # hidden
S_WA = 1024.0       # wq_a / wkv_a
S_A = S_H * S_WA    # stage-A psum scale
S_LAT = 16.0        # latent hi/lo
S_WB = 1024.0       # wq_b / wkv_b
S_B = S_LAT * S_WB  # stage-B psum scale
S_QN = 16.0         # q_nope
S_KN = 32.0         # k_nope
S_QPE = 32.0        # q rope
S_KROT = 16.0       # k rope
S_SC = S_QN * S_KN  # score psum scale (= S_QPE*S_KROT)
S_ATT = 32.0        # attn values
S_WO = 1024.0       # wo
EXP_SCALE = SOFT_SCALE / S_SC

REPLICA_GROUPS = [[0, 1, 2, 3], [4, 5, 6, 7]]


def _yarn_cos_sin(seq_len):
    dim = ROPE
    ar = np.arange(0, dim, 2, dtype=np.float32)
    freq_extra = 1.0 / BASE ** (ar / dim)
    freq_inter = 1.0 / (SCALE * BASE ** (ar / dim))
    low = math.floor(dim * math.log(ORIG_MAX / (BETA_FAST * 2 * math.pi)) / (2 * math.log(BASE)))
    high = math.ceil(dim * math.log(ORIG_MAX / (BETA_SLOW * 2 * math.pi)) / (2 * math.log(BASE)))
    low, high = max(low, 0), min(high, dim - 1)
    denom = (high - low) if high != low else 0.001
    ramp = np.clip((np.arange(dim // 2, dtype=np.float32) - low) / denom, 0.0, 1.0)
    inv_freq_mask = 1.0 - ramp
    inv_freq = freq_inter * (1.0 - inv_freq_mask) + freq_extra * inv_freq_mask
    t = np.arange(seq_len, dtype=np.float32)
    freqs = np.outer(t, inv_freq)
    emb = np.concatenate([freqs, freqs], axis=-1)
    return np.cos(emb).astype(np.float32), np.sin(emb).astype(np.float32)


_PERM64 = np.concatenate([np.arange(0, 64, 2), np.arange(1, 64, 2)])


def _bf16(x):
    return np.ascontiguousarray(x.astype(ml_dtypes.bfloat16))


def _split8(x, scale):
    """x (f32) -> (lo, hi) e4m3 planes at the given scale."""
    xs = np.asarray(x, np.float32) * scale
    hi = xs.astype(E4)
    lo = (xs - hi.astype(np.float32)).astype(E4)
    return lo, hi


def _stack_w(x, scale):
    """weight [K, F] -> [K, 2, F] with planes (lo, hi)."""
    lo, hi = _split8(x, scale)
    return np.ascontiguousarray(np.stack([lo, hi], axis=1))


def _stack_a(x, scale):
    """activation [K, T] -> [K, 2, T] with planes (hi, lo)."""
    lo, hi = _split8(x, scale)
    return np.ascontiguousarray(np.stack([hi, lo], axis=1))


def _emit_stage_a(nc, tc, gin, qcn0, qcn4, krotT, krot4, cosA_sb, sinA_sb,
                  ones_sb, eps_sb, hT8, wqa8d, wkva8d):
    """Down-projections (fp8 DoubleRow 3-product), RMS norms, k-rope.

    Order: the 3 gather slices (chunks 1..3) each run kv+q then launch
    their AllGather immediately; then the 4 replicated chunk-0 quarters;
    then the replicated chunk-4 stage. Latents are emitted as hi/lo fp8."""
    stg_tiles = []
    with (
        tc.tile_pool(name="wA", bufs=1) as wA,
        tc.tile_pool(name="htp", bufs=4) as htp,
        tc.tile_pool(name="stgp", bufs=3) as stgp,
        tc.tile_pool(name="sqp", bufs=12) as sqp,
        tc.tile_pool(name="scrA", bufs=4) as scrA,
        tc.tile_pool(name="psq", bufs=2, space="PSUM") as psq,
        tc.tile_pool(name="pskv", bufs=1, space="PSUM") as pskv,
        tc.tile_pool(name="psnr", bufs=1, space="PSUM") as psnr,
    ):
        wqa_sb = wA.tile([P, KH, 2, Q_LORA], F8, tag="wqa")
        wkva_sb = wA.tile([P, KH, 2, KV_LORA + ROPE], F8, tag="wkva")

        # (hT8 col base, width, gather slice or None, local dest)
        stages = [(512 + sum(w for _, w in _sl_bases[:i]), w, i + 1, None)
                  for i, (_, w) in enumerate(_sl_bases)]
        stages += [(m * P, P, None, ("c0", m)) for m in range(4)]
        stages += [(OWN_T - P, P, None, ("c4", 0))]
        nsl = len(_sl_bases)
        # prefetch the gather slices' tokens, then the weights
        ht_pre = {}
        for i, (colbase, w, _, _) in enumerate(stages[:nsl]):
            ht_pre[colbase] = htp.tile([P, KH, 2, P], F8, tag="ht",
                                       name=f"htp{colbase}")
            for sp in range(2):
                nc.sync.dma_start(
                    ht_pre[colbase][:, :, sp, 0:w],
                    hT8[:, sp, colbase:colbase + w].rearrange(
                        "(k p) t -> p k t", p=P))
            if i == 0:
                for vg in range(2):
                    for sp in range(2):
                        nc.sync.dma_start(
                            wkva_sb[:, :, sp, vg * 288:(vg + 1) * 288],
                            wkva8d[:, sp, vg * 288:(vg + 1) * 288].rearrange(
                                "(k p) f -> p k f", p=P))
        # ordering hacks: wqa DMA waits for wkva + first ht slice
        nc.vector.tensor_copy(wqa_sb[0:1, 0:1, 0:1, 0:1],
                              wkva_sb[0:1, 0:1, 0:1, 0:1])
        nc.vector.tensor_copy(wqa_sb[0:1, 0:1, 0:1, 1:2],
                              ht_pre[stages[2][0]][0:1, 0:1, 0:1, 0:1])
        nc.sync.dma_start(cosA_sb[:], cosA_d[:])
        nc.sync.dma_start(sinA_sb[:], sinA_d[:])
        for wg in range(3):
            for sp in range(2):
                nc.sync.dma_start(
                    wqa_sb[:, :, sp, wg * CW:(wg + 1) * CW],
                    wqa8d[:, sp, wg * CW:(wg + 1) * CW].rearrange(
                        "(k p) f -> p k f", p=P))

        ps_kv = pskv.tile([P, KKV, P], F32, tag="kvps", name="ps_kv_sh")
        ps_nr = psnr.tile([P, 3, P], F32, tag="nrps", name="ps_nr_sh")

        def dr_chain(ps_ap, w_sb, fbase, fw, ht, wd):
            for i in range(KH // 2):
                nc.tensor.matmul(
                    ps_ap, w_sb[:, 2 * i:2 * i + 2, 1, fbase:fbase + fw],
                    ht[:, 2 * i:2 * i + 2, 0, 0:wd],
                    start=(i == 0), stop=False, perf_mode=DR)
            for k in range(KH):
                nc.tensor.matmul(
                    ps_ap, w_sb[:, k, :, fbase:fbase + fw],
                    ht[:, k, :, 0:wd],
                    start=False, stop=(k == KH - 1), perf_mode=DR)

        def emit_kv_phase(ht, wd, dhi, dlo, dkr):
            """kv down-proj chains + norm + rope for one 128-token stage."""
            for f2 in range(KKV):
                dr_chain(ps_kv[:, f2, 0:wd], wkva_sb, f2 * P, P, ht, wd)
            dr_chain(ps_nr[0:ROPE, 2, 0:wd], wkva_sb, KV_LORA, ROPE, ht, wd)
            for f2 in range(KKV):
                sq = sqp.tile([P, P], BF16, tag="sq")
                nc.scalar.activation(sq[:, 0:wd], ps_kv[:, f2, 0:wd],
                                     AF.Square, scale=1.0 / S_A)
                nc.tensor.matmul(ps_nr[:, 1, 0:wd], ones_sb[:], sq[:, 0:wd],
                                 start=(f2 == 0), stop=(f2 == KKV - 1))
            sgkv = scrA.tile([P, P], F32, tag="sgkv")
            nc.scalar.activation(sgkv[:, 0:wd], ps_nr[:, 1, 0:wd], AF.Sqrt,
                                 scale=(S_A / S_LAT) ** 2 / KV_LORA,
                                 bias=eps_sb[:])
            rkv = scrA.tile([P, P], F32, tag="rkv")
            nc.vector.reciprocal(rkv[:, 0:wd], sgkv[:, 0:wd])
            for f2 in range(KKV):
                prod = scrA.tile([P, P], BF16, tag="prod")
                nc.vector.tensor_mul(prod[:, 0:wd], ps_kv[:, f2, 0:wd],
                                     rkv[:, 0:wd])
                nc.scalar.activation(dhi[f2], prod[:, 0:wd], AF.Copy)
                nc.vector.scalar_tensor_tensor(
                    dlo[f2], prod[:, 0:wd], 1.0, dhi[f2], op0=MUL, op1=SUB)

            def rope(colbase):
                ca = cosA_sb[:, colbase:colbase + wd]
                sa = sinA_sb[:, colbase:colbase + wd]
                tmp = scrA.tile([ROPE, P], BF16, tag="tmpr")
                nc.vector.tensor_mul(dkr, ps_nr[0:ROPE, 2, 0:wd], ca)
                nc.vector.tensor_mul(tmp[0:32, 0:wd],
                                     ps_nr[32:ROPE, 2, 0:wd], sa[0:32])
                nc.vector.tensor_mul(tmp[32:ROPE, 0:wd],
                                     ps_nr[0:32, 2, 0:wd], sa[32:ROPE])
                nc.vector.tensor_add(dkr, dkr, tmp[0:ROPE, 0:wd])
            return rope

        def emit_q_phase(ht, wd, dhi, dlo):
            ps_q = psq.tile([P, KQ, P], F32, tag="psq")
            for ft in range(KQ):
                dr_chain(ps_q[:, ft, 0:wd], wqa_sb, ft * P, P, ht, wd)
            for ft in range(KQ):
                sq = sqp.tile([P, P], BF16, tag="sq")
                nc.scalar.activation(sq[:, 0:wd], ps_q[:, ft, 0:wd],
                                     AF.Square, scale=1.0 / S_A)
                nc.tensor.matmul(ps_nr[:, 0, 0:wd], ones_sb[:], sq[:, 0:wd],
                                 start=(ft == 0), stop=(ft == KQ - 1))
            sgq = scrA.tile([P, P], F32, tag="sgq")
            nc.scalar.activation(sgq[:, 0:wd], ps_nr[:, 0, 0:wd], AF.Sqrt,
                                 scale=(S_A / S_LAT) ** 2 / Q_LORA,
                                 bias=eps_sb[:])
            rq = scrA.tile([P, P], F32, tag="rq")
            nc.vector.reciprocal(rq[:, 0:wd], sgq[:, 0:wd])
            for ft in range(KQ):
                prod = scrA.tile([P, P], BF16, tag="prod")
                nc.vector.tensor_mul(prod[:, 0:wd], ps_q[:, ft, 0:wd],
                                     rq[:, 0:wd])
                nc.scalar.activation(dhi[ft], prod[:, 0:wd], AF.Copy)
                nc.vector.scalar_tensor_tensor(
                    dlo[ft], prod[:, 0:wd], 1.0, dhi[ft], op0=MUL, op1=SUB)

        # gather slices first: kv, q, AllGather per slice
        for colbase, wd, gslice, _ in stages[:nsl]:
            ht = ht_pre[colbase]
            stg = stgp.tile([P, NPL, P], F8, tag="stg", name=f"stg{gslice}")
            stg_tiles.append(stg)
            dhi = [stg[:, 2 * (KQ + f2), 0:wd] for f2 in range(KKV)]
            dlo = [stg[:, 2 * (KQ + f2) + 1, 0:wd] for f2 in range(KKV)]
            dkr_scr = scrA.tile([ROPE, P], BF16, tag="dkr",
                                name=f"dkr{gslice}")
            rope = emit_kv_phase(ht, wd, dhi, dlo, dkr_scr[:, 0:wd])
            rope(colbase)
            nc.vector.tensor_copy(
                stg[0:ROPE, 32:34, 0:wd].bitcast(BF16),
                dkr_scr[:, 0:wd].rearrange("p (a b) -> p a b", a=2))
            emit_q_phase(ht, wd,
                         [stg[:, 2 * ft, 0:wd] for ft in range(KQ)],
                         [stg[:, 2 * ft + 1, 0:wd] for ft in range(KQ)])
            nc.sync.dma_start(
                gin[gslice - 1][:].rearrange("f p t -> p f t"),
                stg[:, :, 0:wd])
            nc.gpsimd.collective_compute(
                "AllGather",
                mybir.AluOpType.bypass,
                replica_groups=REPLICA_GROUPS,
                ins=[gin[gslice - 1].opt()],
                outs=[gout_tiles[gslice - 1].opt()],
            )

        # replicated chunk-0 quarters + chunk-4 stage: kv pipelined one
        # stage ahead of q
        qq = []
        for colbase, wd, gslice, dest in stages[nsl:]:
            ht = htp.tile([P, KH, 2, P], F8, tag="ht", name=f"ht{colbase}")
            for sp in range(2):
                nc.sync.dma_start(
                    ht[:, :, sp, 0:wd],
                    hT8[:, sp, colbase:colbase + wd].rearrange(
                        "(k p) t -> p k t", p=P))
            kind, m = dest
            qcn_t = qcn0 if kind == "c0" else qcn4
            m0 = m * P
            dhi = [qcn_t[:, KQ + f2, 0, m0:m0 + P] for f2 in range(KKV)]
            dlo = [qcn_t[:, KQ + f2, 1, m0:m0 + P] for f2 in range(KKV)]
            dkr = krotT[:, m0:m0 + P] if kind == "c0" else krot4[:, 0:P]
            rope = emit_kv_phase(ht, wd, dhi, dlo, dkr)
            rope(colbase)
            qq.append((ht, wd, qcn_t, m0))
            if len(qq) > 1:
                hto, wdo, qt_o, m0o = qq.pop(0)
                emit_q_phase(hto, wdo,
                             [qt_o[:, ft, 0, m0o:m0o + P] for ft in range(KQ)],
                             [qt_o[:, ft, 1, m0o:m0o + P] for ft in range(KQ)])
        for hto, wdo, qt_o, m0o in qq:
            emit_q_phase(hto, wdo,
                         [qt_o[:, ft, 0, m0o:m0o + P] for ft in range(KQ)],
                         [qt_o[:, ft, 1, m0o:m0o + P] for ft in range(KQ)])
    return stg_tiles


def _build_nc():
    global cosA_d, sinA_d
    nc = bacc.Bacc("TRN2", target_bir_lowering=False, debug=False,
                   num_devices=N_CORES)

    hT8 = nc.declare_dram_parameter("hT8", [HIDDEN, 2, OWN_T], F8, isOutput=False)
    wqa8d = nc.declare_dram_parameter("wqa8", [HIDDEN, 2, Q_LORA], F8, isOutput=False)
    wkva8d = nc.declare_dram_parameter("wkva8", [HIDDEN, 2, KV_LORA + ROPE], F8, isOutput=False)
    wqb8d = nc.declare_dram_parameter("wqb8", [Q_LORA, 2, HL * QHD], F8, isOutput=False)
    wkvb8d = nc.declare_dram_parameter("wkvb8", [KV_LORA, 2, HL * (NOPE + VHD)], F8, isOutput=False)
    wo8d = nc.declare_dram_parameter("wo8", [HL * VHD, 2, HIDDEN], F8, isOutput=False)
    cosA_d = nc.declare_dram_parameter("cosA", [ROPE, OWN_T], BF16, isOutput=False)
    sinA_d = nc.declare_dram_parameter("sinA", [ROPE, OWN_T], BF16, isOutput=False)
    cos_tm = nc.declare_dram_parameter("cos_tm", [S, ROPE], BF16, isOutput=False)
    sin_tm_s = nc.declare_dram_parameter("sin_tm_s", [S, ROPE], BF16, isOutput=False)
    masks = nc.declare_dram_parameter("masks", [P, 896], BF16, isOutput=False)
    outT = nc.declare_dram_parameter("outT", [HIDDEN, S], BF16, isOutput=True)

    TT = S // P

    global gout_tiles
    with tile.TileContext(nc) as tc:
        with (
            tc.tile_pool(name="glob", bufs=1) as pp,
            tc.tile_pool(name="dram", bufs=1, space="DRAM") as dramp,
            tc.tile_pool(name="qcnp", bufs=2) as qcnp,
        ):
            krotT = pp.tile([ROPE, CW], BF16, tag="krotT")
            krot_cs = [pp.tile([ROPE, CW], BF16, tag=f"krotc{i}",
                               name=f"krotc{i}")
                       for i in range(NGATH)]
            krot4 = pp.tile([ROPE, P], BF16, tag="krot4")
            costok = pp.tile([P, TT, ROPE], BF16, tag="costok")
            sintok = pp.tile([P, TT, ROPE], BF16, tag="sintok")
            masks_sb = pp.tile([P, 896], BF16, tag="masks")
            cosA_sb = pp.tile([ROPE, OWN_T], BF16, tag="cosA")
            sinA_sb = pp.tile([ROPE, OWN_T], BF16, tag="sinA")
            ones_sb = pp.tile([P, P], BF16, tag="ones")
            oratt_sb = pp.tile([P, 1], BF16, tag="oratt")
            ident_sb = pp.tile([P, P], BF16, tag="ident")
            eps_sb = pp.tile([P, 1], F32, tag="eps")

            gin = [dramp.tile([NPL, P, w // 4], F8, tag=f"gin{i+1}",
                              name=f"gin{i+1}")
                   for i, (_, w) in enumerate(CHUNKS[1:1 + NGATH])]
            gout_tiles = [dramp.tile([4, NPL, P, w // 4], F8,
                                     tag=f"gout{i+1}", name=f"gout{i+1}")
                          for i, (_, w) in enumerate(CHUNKS[1:1 + NGATH])]

            nc.vector.memset(eps_sb[:], (S_A / S_LAT) ** 2 * EPS)
            nc.vector.memset(ones_sb[:], 1.0)
            nc.vector.memset(oratt_sb[:], 1.0 / S_ATT)
            bmasks.make_identity(nc, ident_sb[:])
            warm = pp.tile([P, 1], F32, tag="warm")
            nc.scalar.activation(warm[:], eps_sb[:], AF.Sqrt)

            # chunk-0/4 latents are produced locally by stage A
            qcn0 = qcnp.tile([P, KQ + KKV, 2, CW], F8, tag="qcn", name="qcn0")
            qcn4 = pp.tile([P, KQ + KKV, 2, P], F8, tag="qcn4")

            stg_tiles = _emit_stage_a(
                nc, tc, gin, qcn0, qcn4, krotT, krot4, cosA_sb,
                sinA_sb, ones_sb, eps_sb, hT8, wqa8d, wkva8d)
            # write-write deps keep these bulk table loads off the DMA
            # engines until the gather slices are staged
            last_stg = stg_tiles[0]
            nc.vector.tensor_copy(masks_sb[0:1, 0:1], last_stg[0:1, 0, 0:1])
            nc.vector.tensor_copy(costok[0:1, 0:1, 0:1], masks_sb[0:1, 0:1])
            nc.vector.tensor_copy(sintok[0:1, 0:1, 0:1], masks_sb[0:1, 0:1])
            nc.scalar.dma_start(masks_sb[:], masks[:])
            nc.scalar.dma_start(
                costok[:], cos_tm[:].rearrange("(t p) r -> p t r", p=P))
            nc.scalar.dma_start(
                sintok[:], sin_tm_s[:].rearrange("(t p) r -> p t r", p=P))

            # ====== Stages B + C + D, interleaved per 512-token chunk ======
            with (
                tc.tile_pool(name="persB", bufs=1) as persB,
                tc.tile_pool(name="wB", bufs=1) as wB,
                tc.tile_pool(name="q8p", bufs=2) as q8p,
                tc.tile_pool(name="attnp", bufs=2) as attnp,
                tc.tile_pool(name="rbp", bufs=3) as rbp,
                tc.tile_pool(name="ptp", bufs=22) as ptp,
                tc.tile_pool(name="obp", bufs=4) as obp,
                tc.tile_pool(name="scrB", bufs=4) as scrB,
                tc.tile_pool(name="psC", bufs=3, space="PSUM") as psC,
                tc.tile_pool(name="psB", bufs=3, space="PSUM") as psB,
                tc.tile_pool(name="psT", bufs=1, space="PSUM") as psT,
                tc.tile_pool(name="psDen", bufs=1, space="PSUM") as psDen,
            ):
                # kn8: [128, hi/lo, (4 heads | krot at slot 4), S]
                kn8 = persB.tile([P, 2, HL + 1, S], F8, tag="kn8")
                v_sb = persB.tile([P, HL, TT, VHD], BF16, tag="v")
                # krot pad rows (64:128) of kn8 slot 4 must be zero
                nc.vector.memset(kn8[ROPE:P, :, HL, :], 0.0)

                trslot = [0]
                trt = psT.tile([P, 8, P], BF16, tag="trps", name="trt")

                def tr_tile():
                    sl = trslot[0] % 8
                    trslot[0] += 1
                    return trt[:, sl]

                wqb_sb = wB.tile([P, KQ, 2, HL * QHD], F8, tag="wqb")
                wkvb_sb = wB.tile([P, KKV, 2, HL * (NOPE + VHD)], F8, tag="wkvb")
                wo_sb = wB.tile([P, HL, 2, HIDDEN], F8, tag="wo")
                for sp in range(2):
                    nc.sync.dma_start(
                        wqb_sb[:, :, sp], wqb8d[:, sp].rearrange(
                            "(k p) f -> p k f", p=P))
                    nc.sync.dma_start(
                        wkvb_sb[:, :, sp], wkvb8d[:, sp].rearrange(
                            "(k p) f -> p k f", p=P))
                    nc.sync.dma_start(
                        wo_sb[:, :, sp], wo8d[:, sp].rearrange(
                            "(k p) f -> p k f", p=P))

                chunk_floor_ms = [0.015, 0.05, 0.09, 0.125, 0.14]

                def krot_src(c):
                    if c == 0:
                        return krotT
                    if c == 4:
                        return krot4
                    return krot_cs[c - 1]

                # q8 tiles: [128, hi/lo, head, nope|rope, CW]; rope pad rows
                # (64:128) zeroed once per physical buffer
                q8_bufs = [persB.tile([P, 2, HL, 2, CW], F8, tag=f"q8_{i}",
                                      name=f"q8_{i}") for i in range(2)]
                for qb in q8_bufs:
                    nc.vector.memset(qb[ROPE:P, :, :, 1, :], 0.0)

                for c, (c0, W) in enumerate(CHUNKS):
                    tc.tile_set_cur_wait(chunk_floor_ms[c])
                    NT = W // P
                    KT0 = c0 // P
                    W4 = W // 4
                    nkt = KT0 + NT
                    if c == 0:
                        qcn_c = qcn0
                    elif c == 4:
                        qcn_c = qcn4
                    else:
                        g = gout_tiles[c - 1]
                        qcn_c = qcnp.tile([P, KQ + KKV, 2, CW], F8, tag="qcn",
                                          name=f"qcn{c}")
                        for r in range(4):
                            for sp in range(2):
                                nc.sync.dma_start(
                                    qcn_c[:, KQ:KQ + KKV, sp,
                                          r * W4:(r + 1) * W4],
                                    g[r, 2 * KQ + sp:2 * (KQ + KKV):2]
                                    .rearrange("f p t -> p f t"))
                        for r in range(4):
                            for sp in range(2):
                                nc.sync.dma_start(
                                    qcn_c[:, 0:KQ, sp, r * W4:(r + 1) * W4],
                                    g[r, sp:2 * KQ:2].rearrange(
                                        "f p t -> p f t"))
                        for r in range(4):
                            nc.scalar.dma_start(
                                krot_cs[c - 1][:, r * W4:(r + 1) * W4]
                                .rearrange("p (a b) -> p a b", a=2),
                                g[r, 32:34, 0:ROPE].rearrange(
                                    "a p t -> p a t").bitcast(BF16))

                    # ---- B: up-projections for this chunk ----
                    kn_blks = (None,)

                    def wslices(b0, bw):
                        out = []
                        ws = 0
                        while ws < bw:
                            wd = min(512, bw - ws)
                            out.append((b0 + ws, wd))
                            ws += wd
                        return out

                    def up_chain(ps, lbase, kcnt, w_sb, f0, fw, b0, bw):
                        """psum[:, b0:b0+bw] (+)= latents x w (3-product DR),
                        moving = latents (tokens)."""
                        for ws, wd in wslices(b0, bw):
                            psl = ps[:, ws:ws + wd]
                            for i in range(kcnt // 2):
                                nc.tensor.matmul(
                                    psl,
                                    w_sb[:, 2 * i:2 * i + 2, 1, f0:f0 + fw],
                                    qcn_c[:, lbase + 2 * i:lbase + 2 * i + 2,
                                          0, ws:ws + wd],
                                    start=(i == 0), stop=False, perf_mode=DR)
                            for k in range(kcnt):
                                nc.tensor.matmul(
                                    psl, w_sb[:, k, :, f0:f0 + fw],
                                    qcn_c[:, lbase + k, :, ws:ws + wd],
                                    start=False, stop=(k == kcnt - 1),
                                    perf_mode=DR)

                    # kn chains -> kn8 hi/lo
                    for h in range(HL):
                        for m in kn_blks:
                            b0 = 0 if m is None else m * P
                            bw = W if m is None else P
                            ps = psB.tile([P, CW], F32, tag="bps",
                                          name=f"knps{c}_{h}_{m}")
                            up_chain(ps, KQ, KKV, wkvb_sb, h * P, P, b0, bw)
                            nc.scalar.activation(
                                kn8[:, 0, h, c0 + b0:c0 + b0 + bw],
                                ps[:, b0:b0 + bw], AF.Copy,
                                scale=S_KN / S_B)
                            nc.vector.scalar_tensor_tensor(
                                kn8[:, 1, h, c0 + b0:c0 + b0 + bw],
                                ps[:, b0:b0 + bw], S_KN / S_B,
                                kn8[:, 0, h, c0 + b0:c0 + b0 + bw],
                                op0=MUL, op1=SUB)
                    # krot8 for this chunk: slot HL of kn8 (rows 0:64)
                    ksrc = krot_src(c)
                    nc.scalar.activation(kn8[0:ROPE, 0, HL, c0:c0 + W],
                                         ksrc[:, 0:W], AF.Copy, scale=S_KROT)
                    nc.vector.scalar_tensor_tensor(
                        kn8[0:ROPE, 1, HL, c0:c0 + W], ksrc[:, 0:W], S_KROT,
                        kn8[0:ROPE, 0, HL, c0:c0 + W], op0=MUL, op1=SUB)

                    # v chains: stationary = latents (tokens), moving = w
                    for tt in range(NT):
                        ts = tt * P
                        vc = HL * P
                        pv = psB.tile([P, CW], F32, tag="bps",
                                      name=f"vps{c}_{tt}")
                        for i in range(KKV // 2):
                            nc.tensor.matmul(
                                pv[:],
                                qcn_c[:, KQ + 2 * i:KQ + 2 * i + 2, 0,
                                      ts:ts + P],
                                wkvb_sb[:, 2 * i:2 * i + 2, 1, vc:vc + HL * VHD],
                                start=(i == 0), stop=False, perf_mode=DR)
                        for k in range(KKV):
                            nc.tensor.matmul(
                                pv[:], qcn_c[:, KQ + k, :, ts:ts + P],
                                wkvb_sb[:, k, :, vc:vc + HL * VHD],
                                start=False, stop=(k == KKV - 1),
                                perf_mode=DR)
                        nc.vector.tensor_scalar_mul(
                            v_sb[:, :, KT0 + tt, :],
                            pv[:].rearrange("p (a b) -> p a b", b=VHD),
                            1.0 / S_B)

                    q8_c = q8_bufs[c % 2]
                    # qnope chains -> q8 hi/lo
                    for h in range(HL):
                        for m in kn_blks:
                            b0 = 0 if m is None else m * P
                            bw = W if m is None else P
                            ps = psB.tile([P, CW], F32, tag="bps",
                                          name=f"qnps{c}_{h}_{m}")
                            up_chain(ps, 0, KQ, wqb_sb, h * P, P, b0, bw)
                            nc.scalar.activation(
                                q8_c[:, 0, h, 0, b0:b0 + bw],
                                ps[:, b0:b0 + bw], AF.Copy, scale=S_QN / S_B)
                            nc.vector.scalar_tensor_tensor(
                                q8_c[:, 1, h, 0, b0:b0 + bw],
                                ps[:, b0:b0 + bw], S_QN / S_B,
                                q8_c[:, 0, h, 0, b0:b0 + bw],
                                op0=MUL, op1=SUB)

                    # q-rope: stationary = latents (tokens), moving = w pe
                    for tt in range(NT):
                        ts = tt * P
                        pspe_t = psB.tile([P, CW], F32, tag="bps",
                                          name=f"pspe{c}_{tt}")
                        pspe = pspe_t[:, 0:HL * ROPE]
                        pc = HL * P
                        for i in range(KQ // 2):
                            nc.tensor.matmul(
                                pspe, qcn_c[:, 2 * i:2 * i + 2, 0, ts:ts + P],
                                wqb_sb[:, 2 * i:2 * i + 2, 1,
                                       pc:pc + HL * ROPE],
                                start=(i == 0), stop=False, perf_mode=DR)
                        for k in range(KQ):
                            nc.tensor.matmul(
                                pspe, qcn_c[:, k, :, ts:ts + P],
                                wqb_sb[:, k, :, pc:pc + HL * ROPE],
                                start=False, stop=(k == KQ - 1), perf_mode=DR)
                        t_glob = KT0 + tt
                        qr = scrB.tile([P, HL, ROPE], BF16, tag="qr")
                        qtmp = scrB.tile([P, HL, ROPE], BF16, tag="qtmp")
                        pspe_v = pspe_t[:, 0:HL * ROPE].rearrange(
                            "p (h r) -> p h r", r=ROPE)
                        cos_b = costok[:, t_glob].unsqueeze(1).broadcast_to(
                            [P, HL, ROPE])
                        nc.vector.tensor_mul(qr[:], pspe_v, cos_b)
                        sin_b1 = sintok[:, t_glob, 0:32].unsqueeze(
                            1).broadcast_to([P, HL, 32])
                        sin_b2 = sintok[:, t_glob, 32:ROPE].unsqueeze(
                            1).broadcast_to([P, HL, 32])
                        nc.vector.tensor_mul(qtmp[:, :, 0:32],
                                             pspe_v[:, :, 32:ROPE], sin_b1)
                        nc.vector.tensor_mul(qtmp[:, :, 32:ROPE],
                                             pspe_v[:, :, 0:32], sin_b2)
                        nc.vector.tensor_add(qr[:], qr[:], qtmp[:])
                        for h in range(HL):
                            tr = tr_tile()
                            nc.tensor.transpose(tr[0:ROPE, :], qr[:, h],
                                                ident_sb[:])
                            nc.scalar.activation(
                                q8_c[0:ROPE, 0, h, 1, ts:ts + P],
                                tr[0:ROPE, :], AF.Copy, scale=S_QPE)
                            nc.vector.scalar_tensor_tensor(
                                q8_c[0:ROPE, 1, h, 1, ts:ts + P],
                                tr[0:ROPE, :], S_QPE,
                                q8_c[0:ROPE, 0, h, 1, ts:ts + P],
                                op0=MUL, op1=SUB)

                    # ---- C: attention for this chunk's queries ----
                    attn8 = attnp.tile([P, HL, 2, CW], F8, tag="attn",
                                       name=f"attn{c}")


                    def dr3_scores(ss, h, kt, ps0, qs0, wd):
                        stat_hi = kn8[:, 0, h:HL + 1:HL - h,
                                      kt * P:(kt + 1) * P]
                        stat_lo = kn8[:, 1, h:HL + 1:HL - h,
                                      kt * P:(kt + 1) * P]
                        nc.tensor.matmul(
                            ss[:, ps0:ps0 + wd], stat_hi,
                            q8_c[:, 0, h, :, qs0:qs0 + wd],
                            start=True, stop=False, perf_mode=DR)
                        nc.tensor.matmul(
                            ss[:, ps0:ps0 + wd], stat_lo,
                            q8_c[:, 0, h, :, qs0:qs0 + wd],
                            start=False, stop=False, perf_mode=DR)
                        nc.tensor.matmul(
                            ss[:, ps0:ps0 + wd], stat_hi,
                            q8_c[:, 1, h, :, qs0:qs0 + wd],
                            start=False, stop=True, perf_mode=DR)

                    for h in range(HL):
                        pts = []
                        if W == P:
                            # tail chunk: pack 4 key tiles per psum bank
                            for qd in range(nkt // 4):
                                ss = psC.tile([P, CW], F32, tag="cps")
                                for j in range(4):
                                    kt = qd * 4 + j
                                    dr3_scores(ss, h, kt, j * P, 0, P)
                                pt = ptp.tile([P, CW], BF16, tag="pt")
                                nc.scalar.activation(pt[:], ss[:], AF.Exp,
                                                     scale=EXP_SCALE)
                                for j in range(4):
                                    kt = qd * 4 + j
                                    if kt >= KT0:
                                        nc.vector.tensor_mul(
                                            pt[:, j * P:(j + 1) * P],
                                            pt[:, j * P:(j + 1) * P],
                                            masks_sb[:, 384:384 + P])
                                    pts.append(pt[:, j * P:(j + 1) * P])
                        else:
                            for kt in range(nkt):
                                diag = (kt >= KT0)
                                off = (kt - KT0) * P if diag else 0
                                ss = psC.tile([P, CW], F32, tag="cps")
                                dr3_scores(ss, h, kt, 0, 0, W)
                                pt = ptp.tile([P, CW], BF16, tag="pt")
                                nc.scalar.activation(pt[:, 0:W], ss[:, 0:W],
                                                     AF.Exp, scale=EXP_SCALE)
                                if diag:
                                    nc.vector.tensor_mul(
                                        pt[:, 0:off + P],
                                        pt[:, 0:off + P],
                                        masks_sb[:, 384 - off:384 + P])
                                pts.append(pt[:, 0:W])

                        # PV (v-stationary) + denominators
                        pv_ps = psC.tile([P, CW], F32, tag="cps",
                                         name=f"pv{c}_{h}")
                        for kt in range(nkt):
                            nc.tensor.matmul(
                                pv_ps[:, 0:W], v_sb[:, h, kt, :],
                                pts[kt], start=(kt == 0),
                                stop=(kt == nkt - 1))
                        dn = psDen.tile([1, CW], F32, tag="den",
                                        name=f"dn{c}_{h}")
                        for kt in range(nkt):
                            nc.tensor.matmul(
                                dn[0:1, 0:W], oratt_sb[:], pts[kt],
                                start=(kt == 0), stop=(kt == nkt - 1))
                        rrow = scrB.tile([1, CW], BF16, tag="rrow")
                        with nc.allow_low_precision(reason="softmax denom recip"):
                            nc.vector.reciprocal(rrow[0:1, 0:W], dn[0:1, 0:W])
                        rb = rbp.tile([P, CW], BF16, tag="rb")
                        nc.gpsimd.partition_broadcast(rb[:, 0:W],
                                                      rrow[0:1, 0:W])
                        prod = scrB.tile([P, CW], BF16, tag="pda")
                        nc.vector.tensor_mul(prod[:, 0:W], pv_ps[:, 0:W],
                                             rb[:, 0:W])
                        nc.scalar.activation(attn8[:, h, 0, 0:W],
                                             prod[:, 0:W], AF.Copy)
                        nc.vector.scalar_tensor_tensor(
                            attn8[:, h, 1, 0:W], prod[:, 0:W], 1.0,
                            attn8[:, h, 0, 0:W], op0=MUL, op1=SUB)

                    # ---- D: wo projection for this chunk ----
                    for og in range(4):
                        ob = obp.tile([P, 4, CW], BF16, tag="ob")
                        for oi in range(4):
                            ot = og * 4 + oi
                            ps = psB.tile([P, CW], F32, tag="bps",
                                          name=f"dps{c}_{og}_{oi}")
                            oc = ot * P
                            for ws, wd in wslices(0, W):
                                psl = ps[:, ws:ws + wd]
                                for i in range(HL // 2):
                                    nc.tensor.matmul(
                                        psl,
                                        wo_sb[:, 2 * i:2 * i + 2, 1,
                                              oc:oc + P],
                                        attn8[:, 2 * i:2 * i + 2, 0,
                                              ws:ws + wd],
                                        start=(i == 0), stop=False,
                                        perf_mode=DR)
                                for k in range(HL):
                                    nc.tensor.matmul(
                                        psl, wo_sb[:, k, :, oc:oc + P],
                                        attn8[:, k, :, ws:ws + wd],
                                        start=False, stop=(k == HL - 1),
                                        perf_mode=DR)
                            if oi % 2 == 0:
                                nc.scalar.activation(
                                    ob[:, oi, 0:W], ps[:, 0:W], AF.Copy,
                                    scale=1.0 / (S_ATT * S_WO))
                            else:
                                nc.vector.tensor_scalar_mul(
                                    ob[:, oi, 0:W], ps[:, 0:W],
                                    1.0 / (S_ATT * S_WO))
                        nc.sync.dma_start(
                            outT[og * 4 * P:(og + 1) * 4 * P,
                                 c0:c0 + W].rearrange("(o p) t -> p o t", p=P),
                            ob[:, :, 0:W])

    nc.compile()
    return nc


_NC_CACHE = {}
_LAST_RES = None


def _get_nc(stages="ALL"):
    if "nc" not in _NC_CACHE:
        _NC_CACHE["nc"] = _build_nc()
    return _NC_CACHE["nc"]


def kernel(hidden_states, position_ids, wq_a, q_a_ln_w, wq_b, wkv_a, kv_a_ln_w,
           wkv_b, wo):
    hidden_states = np.asarray(hidden_states, dtype=np.float32)
    position_ids = np.asarray(position_ids)
    wq_a = np.asarray(wq_a, dtype=np.float32)
    wq_b = np.asarray(wq_b, dtype=np.float32)
    wkv_a = np.asarray(wkv_a, dtype=np.float32)
    wkv_b = np.asarray(wkv_b, dtype=np.float32)
    wo = np.asarray(wo, dtype=np.float32)
    # fold RMSNorm elementwise weights into the up-projections (exact)
    wq_b = wq_b * np.asarray(q_a_ln_w, dtype=np.float32)[None, :]
    wkv_b = wkv_b * np.asarray(kv_a_ln_w, dtype=np.float32)[None, :]
    assert hidden_states.shape == (B, S, HIDDEN)

    cos_t, sin_t = _yarn_cos_sin(S)

    # --- weight preprocessing (shared across cores in each batch group) ---
    wqb8_groups = []
    wkvb8_groups = []
    wo8_groups = []
    for g in range(4):
        heads = range(4 * g, 4 * g + 4)
        rows = []
        for h in heads:
            rows.append(np.arange(h * QHD, h * QHD + NOPE))
        pe_rows = []
        for h in heads:
            pe_rows.append(h * QHD + NOPE + _PERM64)
        rows = np.concatenate(rows + pe_rows)
        wqb8_groups.append(_stack_w(wq_b[rows].T, S_WB))

        rows = []
        for h in heads:
            rows.append(np.arange(h * (NOPE + VHD), h * (NOPE + VHD) + NOPE))
        for h in heads:
            rows.append(np.arange(h * (NOPE + VHD) + NOPE, (h + 1) * (NOPE + VHD)))
        rows = np.concatenate(rows)
        wkvb8_groups.append(_stack_w(wkv_b[rows].T, S_WB))

        cols = np.concatenate([np.arange(h * VHD, (h + 1) * VHD) for h in heads])
        wo8_groups.append(_stack_w(wo[:, cols].T, S_WO))

    wqa8 = _stack_w(wq_a.T, S_WA)
    wkva_perm = wkv_a.copy()
    wkva_perm[KV_LORA:] = wkv_a[KV_LORA + _PERM64]
    wkva8 = _stack_w(wkva_perm.T, S_WA)

    x_idx = np.arange(896)[None, :]
    p_idx = np.arange(P)[:, None]
    masks = _bf16((x_idx >= 384 + p_idx).astype(np.float32))

    # --- per-core inputs ---
    in_maps = []
    for c in range(N_CORES):
        beta, g = c // 4, c % 4
        pos = position_ids[beta].astype(np.int64)
        cos_g = cos_t[pos] / S_A  # stage-A k-rope tables fold A psum descale
        sin_g = sin_t[pos] / S_A
        cg = cos_t[pos] / S_B     # q-rope tables fold the B psum descale
        sg = sin_t[pos] / S_B
        sin_s = np.concatenate([-sg[:, :32], sg[:, 32:]], axis=1)

        own = np.concatenate(
            [np.arange(CHUNKS[0][1])] +
            [np.arange(t0 + g * (w // 4), t0 + (g + 1) * (w // 4))
             for t0, w in CHUNKS[1:1 + NGATH]] +
            [np.arange(CHUNKS[4][0], CHUNKS[4][0] + CHUNKS[4][1])])
        hT8v = _stack_a(hidden_states[beta].T[:, own], S_H)
        cosA = _bf16(cos_g[own].T)
        sg_own = sin_g[own]
        sinA = _bf16(np.concatenate([-sg_own[:, :32].T, sg_own[:, 32:].T], axis=0))

        in_maps.append({
            "hT8": hT8v,
            "wqa8": wqa8,
            "wkva8": wkva8,
            "wqb8": wqb8_groups[g],
            "wkvb8": wkvb8_groups[g],
            "wo8": wo8_groups[g],
            "cosA": cosA,
            "sinA": sinA,
            "cos_tm": _bf16(cg),
            "sin_tm_s": _bf16(sin_s),
            "masks": masks,
        })

    nc = _get_nc()
    global _LAST_RES
    res = run_bass_kernel_spmd(nc, in_maps, core_ids=list(range(N_CORES)))
    _LAST_RES = res

    out = np.zeros((B, S, HIDDEN), dtype=np.float32)
    for c in range(N_CORES):
        out[c // 4] += res.results[c]["outT"].astype(np.float32).T
    return out


# revision 24
# speedup vs baseline: 1.0395x; 1.0395x over previous
"""DeepseekV3 MLA attention forward on 8 Trainium2 NeuronCores.

Sharding: core c -> batch c//4, head group c%4 (4 of 16 heads).

v3: all five GEMM stages run as fp8(e4m3) DoubleRow matmuls with hi/lo
error compensation (3-product scheme: hi*whi + hi*wlo + lo*whi), which the
cost model prices at 0.5 cycles/row -- 2x the bf16 MAC rate for the same
accuracy class as bf16. Weights and the hidden input are split on the host;
device-computed intermediates (latents, q, kn, attn) are split with one Act
copy (hi) + one DVE scalar_tensor_tensor (lo) from the f32 PSUM product.

Attention PV is restructured v-stationary: out[vd, q] accumulates over key
tiles in a single PSUM bank per (head, chunk), so the per-q-tile reciprocal/
scale/transpose chains of v2 disappear. Softmax denominators come from
1-column ones matmuls per (q-tile, key-tile), reciprocals are transposed and
partition-broadcast, and one DVE multiply produces the normalized attn tile
(directly in the [vd, token] layout stage D consumes -- no transposes).

Score tiles are stored unshifted (diagonal tiles keep their causal zero
region) so PV and denominator matmuls are uniform full-width accumulations.

Stage A remains token-sharded across the 4 cores of each batch group with
chunk 0 replicated; per-chunk AllGathers carry the latents as hi/lo fp8
planes (same bytes as the old bf16 payload) plus the bf16 rope key.
"""

import math

import numpy as np
import ml_dtypes

import concourse.bass as bass
import concourse.tile as tile
import concourse.mybir as mybir
from concourse import bacc
from concourse import masks as bmasks
from concourse.bass_utils import run_bass_kernel_spmd

BF16 = mybir.dt.bfloat16
F32 = mybir.dt.float32
F8 = mybir.dt.float8e4
E4 = ml_dtypes.float8_e4m3
AF = mybir.ActivationFunctionType
DR = mybir.MatmulPerfMode.DoubleRow
MUL = mybir.AluOpType.mult
SUB = mybir.AluOpType.subtract

# ---- model config (hardcoded to match the problem spec) ----
HIDDEN = 2048
N_HEADS = 16
Q_LORA = 1536
KV_LORA = 512
NOPE = 128
ROPE = 64
VHD = 128
QHD = NOPE + ROPE  # 192
BASE = 10000.0
SCALE = 40.0
ORIG_MAX = 4096
BETA_FAST = 32
BETA_SLOW = 1
EPS = 1e-6
B = 2
S = 2048

N_CORES = 8
HL = 4           # heads per core
P = 128
KH = HIDDEN // P     # 16
KQ = Q_LORA // P     # 12
KKV = KV_LORA // P   # 4
NPL = 34             # gather planes: 16 ft x (hi,lo) fp8 + krot bf16 (2B)
CW = 512             # max chunk width / psum tile width
CHUNKS = [(0, 512), (512, 512), (1024, 512), (1536, 384), (1920, 128)]
NGATH = 3            # chunks 1..3 gathered; chunks 0 and 4 replicated
_sl_bases = []
_acc = 512
for _, w in CHUNKS[1:1 + NGATH]:
    _sl_bases.append((_acc, w // 4))
    _acc += w // 4
OWN_T = 512 + sum(w for _, w in CHUNKS[1:1 + NGATH]) // 4 + CHUNKS[4][1]  # 992

_m = 0.1 * math.log(SCALE) + 1.0
SOFT_SCALE = (QHD ** -0.5) * _m * _m

# fp8 scales
S_H = 16.0          # hidden
S_WA = 1024.0       # wq_a / wkv_a
S_A = S_H * S_WA    # stage-A psum scale
S_LAT = 16.0        # latent hi/lo
S_WB = 1024.0       # wq_b / wkv_b
S_B = S_LAT * S_WB  # stage-B psum scale
S_QN = 16.0         # q_nope
S_KN = 32.0         # k_nope
S_QPE = 32.0        # q rope
S_KROT = 16.0       # k rope
S_SC = S_QN * S_KN  # score psum scale (= S_QPE*S_KROT)
S_ATT = 32.0        # attn values
S_WO = 1024.0       # wo
EXP_SCALE = SOFT_SCALE / S_SC

REPLICA_GROUPS = [[0, 1, 2, 3], [4, 5, 6, 7]]


def _yarn_cos_sin(seq_len):
    dim = ROPE
    ar = np.arange(0, dim, 2, dtype=np.float32)
    freq_extra = 1.0 / BASE ** (ar / dim)
    freq_inter = 1.0 / (SCALE * BASE ** (ar / dim))
    low = math.floor(dim * math.log(ORIG_MAX / (BETA_FAST * 2 * math.pi)) / (2 * math.log(BASE)))
    high = math.ceil(dim * math.log(ORIG_MAX / (BETA_SLOW * 2 * math.pi)) / (2 * math.log(BASE)))
    low, high = max(low, 0), min(high, dim - 1)
    denom = (high - low) if high != low else 0.001
    ramp = np.clip((np.arange(dim // 2, dtype=np.float32) - low) / denom, 0.0, 1.0)
    inv_freq_mask = 1.0 - ramp
    inv_freq = freq_inter * (1.0 - inv_freq_mask) + freq_extra * inv_freq_mask
    t = np.arange(seq_len, dtype=np.float32)
    freqs = np.outer(t, inv_freq)
    emb = np.concatenate([freqs, freqs], axis=-1)
    return np.cos(emb).astype(np.float32), np.sin(emb).astype(np.float32)


_PERM64 = np.concatenate([np.arange(0, 64, 2), np.arange(1, 64, 2)])


def _bf16(x):
    return np.ascontiguousarray(x.astype(ml_dtypes.bfloat16))


def _split8(x, scale):
    """x (f32) -> (lo, hi) e4m3 planes at the given scale."""
    xs = np.asarray(x, np.float32) * scale
    hi = xs.astype(E4)
    lo = (xs - hi.astype(np.float32)).astype(E4)
    return lo, hi


def _stack_w(x, scale):
    """weight [K, F] -> [K, 2, F] with planes (lo, hi)."""
    lo, hi = _split8(x, scale)
    return np.ascontiguousarray(np.stack([lo, hi], axis=1))


def _stack_a(x, scale):
    """activation [K, T] -> [K, 2, T] with planes (hi, lo)."""
    lo, hi = _split8(x, scale)
    return np.ascontiguousarray(np.stack([hi, lo], axis=1))


def _emit_stage_a(nc, tc, gin, qcn0, qcn4, krotT, krot4, cosA_sb, sinA_sb,
                  ones_sb, eps_sb, hT8, wqa8d, wkva8d):
    """Down-projections (fp8 DoubleRow 3-product), RMS norms, k-rope.

    Order: the 3 gather slices (chunks 1..3) each run kv+q then launch
    their AllGather immediately; then the 4 replicated chunk-0 quarters;
    then the replicated chunk-4 stage. Latents are emitted as hi/lo fp8."""
    stg_tiles = []
    with (
        tc.tile_pool(name="wA", bufs=1) as wA,
        tc.tile_pool(name="htp", bufs=4) as htp,
        tc.tile_pool(name="stgp", bufs=3) as stgp,
        tc.tile_pool(name="sqp", bufs=12) as sqp,
        tc.tile_pool(name="scrA", bufs=4) as scrA,
        tc.tile_pool(name="psq", bufs=2, space="PSUM") as psq,
        tc.tile_pool(name="pskv", bufs=1, space="PSUM") as pskv,
        tc.tile_pool(name="psnr", bufs=1, space="PSUM") as psnr,
    ):
        wqa_sb = wA.tile([P, KH, 2, Q_LORA], F8, tag="wqa")
        wkva_sb = wA.tile([P, KH, 2, KV_LORA + ROPE], F8, tag="wkva")

        # (hT8 col base, width, gather slice or None, local dest)
        stages = [(512 + sum(w for _, w in _sl_bases[:i]), w, i + 1, None)
                  for i, (_, w) in enumerate(_sl_bases)]
        stages += [(m * P, P, None, ("c0", m)) for m in range(4)]
        stages += [(OWN_T - P, P, None, ("c4", 0))]
        nsl = len(_sl_bases)
        # prefetch the gather slices' tokens, then the weights
        ht_pre = {}
        for i, (colbase, w, _, _) in enumerate(stages[:nsl]):
            ht_pre[colbase] = htp.tile([P, KH, 2, P], F8, tag="ht",
                                       name=f"htp{colbase}")
            for sp in range(2):
                nc.sync.dma_start(
                    ht_pre[colbase][:, :, sp, 0:w],
                    hT8[:, sp, colbase:colbase + w].rearrange(
                        "(k p) t -> p k t", p=P))
            if i == 0:
                for vg in range(2):
                    for sp in range(2):
                        nc.sync.dma_start(
                            wkva_sb[:, :, sp, vg * 288:(vg + 1) * 288],
                            wkva8d[:, sp, vg * 288:(vg + 1) * 288].rearrange(
                                "(k p) f -> p k f", p=P))
        # ordering hacks: wqa DMA waits for wkva + first ht slice
        nc.vector.tensor_copy(wqa_sb[0:1, 0:1, 0:1, 0:1],
                              wkva_sb[0:1, 0:1, 0:1, 0:1])
        nc.vector.tensor_copy(wqa_sb[0:1, 0:1, 0:1, 1:2],
                              ht_pre[stages[2][0]][0:1, 0:1, 0:1, 0:1])
        nc.sync.dma_start(cosA_sb[:], cosA_d[:])
        nc.sync.dma_start(sinA_sb[:], sinA_d[:])
        for wg in range(3):
            for sp in range(2):
                nc.sync.dma_start(
                    wqa_sb[:, :, sp, wg * CW:(wg + 1) * CW],
                    wqa8d[:, sp, wg * CW:(wg + 1) * CW].rearrange(
                        "(k p) f -> p k f", p=P))

        ps_kv = pskv.tile([P, KKV, P], F32, tag="kvps", name="ps_kv_sh")
        ps_nr = psnr.tile([P, 3, P], F32, tag="nrps", name="ps_nr_sh")

        def dr_chain(ps_ap, w_sb, fbase, fw, ht, wd):
            for i in range(KH // 2):
                nc.tensor.matmul(
                    ps_ap, w_sb[:, 2 * i:2 * i + 2, 1, fbase:fbase + fw],
                    ht[:, 2 * i:2 * i + 2, 0, 0:wd],
                    start=(i == 0), stop=False, perf_mode=DR)
            for k in range(KH):
                nc.tensor.matmul(
                    ps_ap, w_sb[:, k, :, fbase:fbase + fw],
                    ht[:, k, :, 0:wd],
                    start=False, stop=(k == KH - 1), perf_mode=DR)

        def emit_kv_phase(ht, wd, dhi, dlo, dkr):
            """kv down-proj chains + norm + rope for one 128-token stage."""
            for f2 in range(KKV):
                dr_chain(ps_kv[:, f2, 0:wd], wkva_sb, f2 * P, P, ht, wd)
            dr_chain(ps_nr[0:ROPE, 2, 0:wd], wkva_sb, KV_LORA, ROPE, ht, wd)
            for f2 in range(KKV):
                sq = sqp.tile([P, P], BF16, tag="sq")
                nc.scalar.activation(sq[:, 0:wd], ps_kv[:, f2, 0:wd],
                                     AF.Square, scale=1.0 / S_A)
                nc.tensor.matmul(ps_nr[:, 1, 0:wd], ones_sb[:], sq[:, 0:wd],
                                 start=(f2 == 0), stop=(f2 == KKV - 1))
            sgkv = scrA.tile([P, P], F32, tag="sgkv")
            nc.scalar.activation(sgkv[:, 0:wd], ps_nr[:, 1, 0:wd], AF.Sqrt,
                                 scale=(S_A / S_LAT) ** 2 / KV_LORA,
                                 bias=eps_sb[:])
            rkv = scrA.tile([P, P], F32, tag="rkv")
            nc.vector.reciprocal(rkv[:, 0:wd], sgkv[:, 0:wd])
            for f2 in range(KKV):
                prod = scrA.tile([P, P], BF16, tag="prod")
                nc.vector.tensor_mul(prod[:, 0:wd], ps_kv[:, f2, 0:wd],
                                     rkv[:, 0:wd])
                nc.scalar.activation(dhi[f2], prod[:, 0:wd], AF.Copy)
                nc.vector.scalar_tensor_tensor(
                    dlo[f2], prod[:, 0:wd], 1.0, dhi[f2], op0=MUL, op1=SUB)

            def rope(colbase):
                ca = cosA_sb[:, colbase:colbase + wd]
                sa = sinA_sb[:, colbase:colbase + wd]
                tmp = scrA.tile([ROPE, P], BF16, tag="tmpr")
                nc.vector.tensor_mul(dkr, ps_nr[0:ROPE, 2, 0:wd], ca)
                nc.vector.tensor_mul(tmp[0:32, 0:wd],
                                     ps_nr[32:ROPE, 2, 0:wd], sa[0:32])
                nc.vector.tensor_mul(tmp[32:ROPE, 0:wd],
                                     ps_nr[0:32, 2, 0:wd], sa[32:ROPE])
                nc.vector.tensor_add(dkr, dkr, tmp[0:ROPE, 0:wd])
            return rope

        def emit_q_phase(ht, wd, dhi, dlo):
            ps_q = psq.tile([P, KQ, P], F32, tag="psq")
            for ft in range(KQ):
                dr_chain(ps_q[:, ft, 0:wd], wqa_sb, ft * P, P, ht, wd)
            for ft in range(KQ):
                sq = sqp.tile([P, P], BF16, tag="sq")
                nc.scalar.activation(sq[:, 0:wd], ps_q[:, ft, 0:wd],
                                     AF.Square, scale=1.0 / S_A)
                nc.tensor.matmul(ps_nr[:, 0, 0:wd], ones_sb[:], sq[:, 0:wd],
                                 start=(ft == 0), stop=(ft == KQ - 1))
            sgq = scrA.tile([P, P], F32, tag="sgq")
            nc.scalar.activation(sgq[:, 0:wd], ps_nr[:, 0, 0:wd], AF.Sqrt,
                                 scale=(S_A / S_LAT) ** 2 / Q_LORA,
                                 bias=eps_sb[:])
            rq = scrA.tile([P, P], F32, tag="rq")
            nc.vector.reciprocal(rq[:, 0:wd], sgq[:, 0:wd])
            for ft in range(KQ):
                prod = scrA.tile([P, P], BF16, tag="prod")
                nc.vector.tensor_mul(prod[:, 0:wd], ps_q[:, ft, 0:wd],
                                     rq[:, 0:wd])
                nc.scalar.activation(dhi[ft], prod[:, 0:wd], AF.Copy)
                nc.vector.scalar_tensor_tensor(
                    dlo[ft], prod[:, 0:wd], 1.0, dhi[ft], op0=MUL, op1=SUB)

        # gather slices first: kv, q, AllGather per slice
        for colbase, wd, gslice, _ in stages[:nsl]:
            ht = ht_pre[colbase]
            stg = stgp.tile([P, NPL, P], F8, tag="stg", name=f"stg{gslice}")
            stg_tiles.append(stg)
            dhi = [stg[:, 2 * (KQ + f2), 0:wd] for f2 in range(KKV)]
            dlo = [stg[:, 2 * (KQ + f2) + 1, 0:wd] for f2 in range(KKV)]
            dkr_scr = scrA.tile([ROPE, P], BF16, tag="dkr",
                                name=f"dkr{gslice}")
            rope = emit_kv_phase(ht, wd, dhi, dlo, dkr_scr[:, 0:wd])
            rope(colbase)
            nc.vector.tensor_copy(
                stg[0:ROPE, 32:34, 0:wd].bitcast(BF16),
                dkr_scr[:, 0:wd].rearrange("p (a b) -> p a b", a=2))
            emit_q_phase(ht, wd,
                         [stg[:, 2 * ft, 0:wd] for ft in range(KQ)],
                         [stg[:, 2 * ft + 1, 0:wd] for ft in range(KQ)])
            nc.sync.dma_start(
                gin[gslice - 1][:].rearrange("f p t -> p f t"),
                stg[:, :, 0:wd])
            nc.gpsimd.collective_compute(
                "AllGather",
                mybir.AluOpType.bypass,
                replica_groups=REPLICA_GROUPS,
                ins=[gin[gslice - 1].opt()],
                outs=[gout_tiles[gslice - 1].opt()],
            )

        # replicated chunk-0 quarters + chunk-4 stage: kv pipelined one
        # stage ahead of q
        qq = []
        for colbase, wd, gslice, dest in stages[nsl:]:
            ht = htp.tile([P, KH, 2, P], F8, tag="ht", name=f"ht{colbase}")
            for sp in range(2):
                nc.sync.dma_start(
                    ht[:, :, sp, 0:wd],
                    hT8[:, sp, colbase:colbase + wd].rearrange(
                        "(k p) t -> p k t", p=P))
            kind, m = dest
            qcn_t = qcn0 if kind == "c0" else qcn4
            m0 = m * P
            dhi = [qcn_t[:, KQ + f2, 0, m0:m0 + P] for f2 in range(KKV)]
            dlo = [qcn_t[:, KQ + f2, 1, m0:m0 + P] for f2 in range(KKV)]
            dkr = krotT[:, m0:m0 + P] if kind == "c0" else krot4[:, 0:P]
            rope = emit_kv_phase(ht, wd, dhi, dlo, dkr)
            rope(colbase)
            qq.append((ht, wd, qcn_t, m0))
            if len(qq) > 1:
                hto, wdo, qt_o, m0o = qq.pop(0)
                emit_q_phase(hto, wdo,
                             [qt_o[:, ft, 0, m0o:m0o + P] for ft in range(KQ)],
                             [qt_o[:, ft, 1, m0o:m0o + P] for ft in range(KQ)])
        for hto, wdo, qt_o, m0o in qq:
            emit_q_phase(hto, wdo,
                         [qt_o[:, ft, 0, m0o:m0o + P] for ft in range(KQ)],
                         [qt_o[:, ft, 1, m0o:m0o + P] for ft in range(KQ)])
    return stg_tiles


def _build_nc():
    global cosA_d, sinA_d
    nc = bacc.Bacc("TRN2", target_bir_lowering=False, debug=False,
                   num_devices=N_CORES)

    hT8 = nc.declare_dram_parameter("hT8", [HIDDEN, 2, OWN_T], F8, isOutput=False)
    wqa8d = nc.declare_dram_parameter("wqa8", [HIDDEN, 2, Q_LORA], F8, isOutput=False)
    wkva8d = nc.declare_dram_parameter("wkva8", [HIDDEN, 2, KV_LORA + ROPE], F8, isOutput=False)
    wqb8d = nc.declare_dram_parameter("wqb8", [Q_LORA, 2, HL * QHD], F8, isOutput=False)
    wkvb8d = nc.declare_dram_parameter("wkvb8", [KV_LORA, 2, HL * (NOPE + VHD)], F8, isOutput=False)
    wo8d = nc.declare_dram_parameter("wo8", [HL * VHD, 2, HIDDEN], F8, isOutput=False)
    cosA_d = nc.declare_dram_parameter("cosA", [ROPE, OWN_T], BF16, isOutput=False)
    sinA_d = nc.declare_dram_parameter("sinA", [ROPE, OWN_T], BF16, isOutput=False)
    cos_tm = nc.declare_dram_parameter("cos_tm", [S, ROPE], BF16, isOutput=False)
    sin_tm_s = nc.declare_dram_parameter("sin_tm_s", [S, ROPE], BF16, isOutput=False)
    masks = nc.declare_dram_parameter("masks", [P, 896], BF16, isOutput=False)
    outT = nc.declare_dram_parameter("outT", [HIDDEN, S], BF16, isOutput=True)

    TT = S // P

    global gout_tiles
    with tile.TileContext(nc) as tc:
        with (
            tc.tile_pool(name="glob", bufs=1) as pp,
            tc.tile_pool(name="dram", bufs=1, space="DRAM") as dramp,
            tc.tile_pool(name="qcnp", bufs=2) as qcnp,
        ):
            krotT = pp.tile([ROPE, CW], BF16, tag="krotT")
            krot_cs = [pp.tile([ROPE, CW], BF16, tag=f"krotc{i}",
                               name=f"krotc{i}")
                       for i in range(NGATH)]
            krot4 = pp.tile([ROPE, P], BF16, tag="krot4")
            costok = pp.tile([P, TT, ROPE], BF16, tag="costok")
            sintok = pp.tile([P, TT, ROPE], BF16, tag="sintok")
            masks_sb = pp.tile([P, 896], BF16, tag="masks")
            cosA_sb = pp.tile([ROPE, OWN_T], BF16, tag="cosA")
            sinA_sb = pp.tile([ROPE, OWN_T], BF16, tag="sinA")
            ones_sb = pp.tile([P, P], BF16, tag="ones")
            oratt_sb = pp.tile([P, 1], BF16, tag="oratt")
            ident_sb = pp.tile([P, P], BF16, tag="ident")
            eps_sb = pp.tile([P, 1], F32, tag="eps")

            gin = [dramp.tile([NPL, P, w // 4], F8, tag=f"gin{i+1}",
                              name=f"gin{i+1}")
                   for i, (_, w) in enumerate(CHUNKS[1:1 + NGATH])]
            gout_tiles = [dramp.tile([4, NPL, P, w // 4], F8,
                                     tag=f"gout{i+1}", name=f"gout{i+1}")
                          for i, (_, w) in enumerate(CHUNKS[1:1 + NGATH])]

            nc.vector.memset(eps_sb[:], (S_A / S_LAT) ** 2 * EPS)
            nc.vector.memset(ones_sb[:], 1.0)
            nc.vector.memset(oratt_sb[:], 1.0 / S_ATT)
            bmasks.make_identity(nc, ident_sb[:])
            warm = pp.tile([P, 1], F32, tag="warm")
            nc.scalar.activation(warm[:], eps_sb[:], AF.Sqrt)

            # chunk-0/4 latents are produced locally by stage A
            qcn0 = qcnp.tile([P, KQ + KKV, 2, CW], F8, tag="qcn", name="qcn0")
            qcn4 = pp.tile([P, KQ + KKV, 2, P], F8, tag="qcn4")

            stg_tiles = _emit_stage_a(
                nc, tc, gin, qcn0, qcn4, krotT, krot4, cosA_sb,
                sinA_sb, ones_sb, eps_sb, hT8, wqa8d, wkva8d)
            # write-write deps keep these bulk table loads off the DMA
            # engines until the gather slices are staged
            last_stg = stg_tiles[0]
            nc.vector.tensor_copy(masks_sb[0:1, 0:1], last_stg[0:1, 0, 0:1])
            nc.vector.tensor_copy(costok[0:1, 0:1, 0:1], masks_sb[0:1, 0:1])
            nc.vector.tensor_copy(sintok[0:1, 0:1, 0:1], masks_sb[0:1, 0:1])
            nc.scalar.dma_start(masks_sb[:], masks[:])
            nc.scalar.dma_start(
                costok[:], cos_tm[:].rearrange("(t p) r -> p t r", p=P))
            nc.scalar.dma_start(
                sintok[:], sin_tm_s[:].rearrange("(t p) r -> p t r", p=P))

            # ====== Stages B + C + D, interleaved per 512-token chunk ======
            with (
                tc.tile_pool(name="persB", bufs=1) as persB,
                tc.tile_pool(name="wB", bufs=1) as wB,
                tc.tile_pool(name="q8p", bufs=2) as q8p,
                tc.tile_pool(name="attnp", bufs=2) as attnp,
                tc.tile_pool(name="rbp", bufs=3) as rbp,
                tc.tile_pool(name="ptp", bufs=22) as ptp,
                tc.tile_pool(name="obp", bufs=4) as obp,
                tc.tile_pool(name="scrB", bufs=4) as scrB,
                tc.tile_pool(name="psC", bufs=4, space="PSUM") as psC,
                tc.tile_pool(name="psB", bufs=3, space="PSUM") as psB,
                tc.tile_pool(name="psT", bufs=1, space="PSUM") as psT,
            ):
                # kn8: [128, hi/lo, (4 heads | krot at slot 4), S]
                kn8 = persB.tile([P, 2, HL + 1, S], F8, tag="kn8")
                v_sb = persB.tile([P, HL, TT, VHD], BF16, tag="v")
                # krot pad rows (64:128) of kn8 slot 4 must be zero
                nc.vector.memset(kn8[ROPE:P, :, HL, :], 0.0)

                trslot = [0]
                trt = psT.tile([P, 8, P], BF16, tag="trps", name="trt")

                def tr_tile():
                    sl = trslot[0] % 8
                    trslot[0] += 1
                    return trt[:, sl]

                wqb_sb = wB.tile([P, KQ, 2, HL * QHD], F8, tag="wqb")
                wkvb_sb = wB.tile([P, KKV, 2, HL * (NOPE + VHD)], F8, tag="wkvb")
                wo_sb = wB.tile([P, HL, 2, HIDDEN], F8, tag="wo")
                for sp in range(2):
                    nc.sync.dma_start(
                        wqb_sb[:, :, sp], wqb8d[:, sp].rearrange(
                            "(k p) f -> p k f", p=P))
                    nc.sync.dma_start(
                        wkvb_sb[:, :, sp], wkvb8d[:, sp].rearrange(
                            "(k p) f -> p k f", p=P))
                    nc.sync.dma_start(
                        wo_sb[:, :, sp], wo8d[:, sp].rearrange(
                            "(k p) f -> p k f", p=P))

                chunk_floor_ms = [0.015, 0.05, 0.09, 0.125, 0.14]

                def krot_src(c):
                    if c == 0:
                        return krotT
                    if c == 4:
                        return krot4
                    return krot_cs[c - 1]

                # q8 tiles: [128, hi/lo, head, nope|rope, CW]; rope pad rows
                # (64:128) zeroed once per physical buffer
                q8_bufs = [persB.tile([P, 2, HL, 2, CW], F8, tag=f"q8_{i}",
                                      name=f"q8_{i}") for i in range(2)]
                for qb in q8_bufs:
                    nc.vector.memset(qb[ROPE:P, :, :, 1, :], 0.0)

                for c, (c0, W) in enumerate(CHUNKS):
                    tc.tile_set_cur_wait(chunk_floor_ms[c], enable=False)
                    NT = W // P
                    KT0 = c0 // P
                    W4 = W // 4
                    nkt = KT0 + NT
                    if c == 0:
                        qcn_c = qcn0
                    elif c == 4:
                        qcn_c = qcn4
                    else:
                        g = gout_tiles[c - 1]
                        qcn_c = qcnp.tile([P, KQ + KKV, 2, CW], F8, tag="qcn",
                                          name=f"qcn{c}")
                        for r in range(4):
                            for sp in range(2):
                                nc.sync.dma_start(
                                    qcn_c[:, KQ:KQ + KKV, sp,
                                          r * W4:(r + 1) * W4],
                                    g[r, 2 * KQ + sp:2 * (KQ + KKV):2]
                                    .rearrange("f p t -> p f t"))
                        for r in range(4):
                            for sp in range(2):
                                nc.sync.dma_start(
                                    qcn_c[:, 0:KQ, sp, r * W4:(r + 1) * W4],
                                    g[r, sp:2 * KQ:2].rearrange(
                                        "f p t -> p f t"))
                        for r in range(4):
                            nc.scalar.dma_start(
                                krot_cs[c - 1][:, r * W4:(r + 1) * W4]
                                .rearrange("p (a b) -> p a b", a=2),
                                g[r, 32:34, 0:ROPE].rearrange(
                                    "a p t -> p a t").bitcast(BF16))

                    # ---- B: up-projections for this chunk ----
                    kn_blks = (None,)

                    def wslices(b0, bw):
                        out = []
                        ws = 0
                        while ws < bw:
                            wd = min(512, bw - ws)
                            out.append((b0 + ws, wd))
                            ws += wd
                        return out

                    def up_chain(ps, lbase, kcnt, w_sb, f0, fw, b0, bw):
                        """psum[:, b0:b0+bw] (+)= latents x w (3-product DR),
                        moving = latents (tokens)."""
                        for ws, wd in wslices(b0, bw):
                            psl = ps[:, ws:ws + wd]
                            for i in range(kcnt // 2):
                                nc.tensor.matmul(
                                    psl,
                                    w_sb[:, 2 * i:2 * i + 2, 1, f0:f0 + fw],
                                    qcn_c[:, lbase + 2 * i:lbase + 2 * i + 2,
                                          0, ws:ws + wd],
                                    start=(i == 0), stop=False, perf_mode=DR)
                            for k in range(kcnt):
                                nc.tensor.matmul(
                                    psl, w_sb[:, k, :, f0:f0 + fw],
                                    qcn_c[:, lbase + k, :, ws:ws + wd],
                                    start=False, stop=(k == kcnt - 1),
                                    perf_mode=DR)

                    # kn chains -> kn8 hi/lo
                    for h in range(HL):
                        for m in kn_blks:
                            b0 = 0 if m is None else m * P
                            bw = W if m is None else P
                            ps = psB.tile([P, CW], F32, tag="bps",
                                          name=f"knps{c}_{h}_{m}")
                            up_chain(ps, KQ, KKV, wkvb_sb, h * P, P, b0, bw)
                            nc.scalar.activation(
                                kn8[:, 0, h, c0 + b0:c0 + b0 + bw],
                                ps[:, b0:b0 + bw], AF.Copy,
                                scale=S_KN / S_B)
                            nc.vector.scalar_tensor_tensor(
                                kn8[:, 1, h, c0 + b0:c0 + b0 + bw],
                                ps[:, b0:b0 + bw], S_KN / S_B,
                                kn8[:, 0, h, c0 + b0:c0 + b0 + bw],
                                op0=MUL, op1=SUB)
                    # krot8 for this chunk: slot HL of kn8 (rows 0:64)
                    ksrc = krot_src(c)
                    nc.scalar.activation(kn8[0:ROPE, 0, HL, c0:c0 + W],
                                         ksrc[:, 0:W], AF.Copy, scale=S_KROT)
                    nc.vector.scalar_tensor_tensor(
                        kn8[0:ROPE, 1, HL, c0:c0 + W], ksrc[:, 0:W], S_KROT,
                        kn8[0:ROPE, 0, HL, c0:c0 + W], op0=MUL, op1=SUB)

                    # v chains: stationary = latents (tokens), moving = w
                    for tt in range(NT):
                        ts = tt * P
                        vc = HL * P
                        pv = psB.tile([P, CW], F32, tag="bps",
                                      name=f"vps{c}_{tt}")
                        for i in range(KKV // 2):
                            nc.tensor.matmul(
                                pv[:],
                                qcn_c[:, KQ + 2 * i:KQ + 2 * i + 2, 0,
                                      ts:ts + P],
                                wkvb_sb[:, 2 * i:2 * i + 2, 1, vc:vc + HL * VHD],
                                start=(i == 0), stop=False, perf_mode=DR)
                        for k in range(KKV):
                            nc.tensor.matmul(
                                pv[:], qcn_c[:, KQ + k, :, ts:ts + P],
                                wkvb_sb[:, k, :, vc:vc + HL * VHD],
                                start=False, stop=(k == KKV - 1),
                                perf_mode=DR)
                        nc.vector.tensor_scalar_mul(
                            v_sb[:, :, KT0 + tt, :],
                            pv[:].rearrange("p (a b) -> p a b", b=VHD),
                            1.0 / S_B)

                    q8_c = q8_bufs[c % 2]
                    # qnope chains -> q8 hi/lo
                    for h in range(HL):
                        for m in kn_blks:
                            b0 = 0 if m is None else m * P
                            bw = W if m is None else P
                            ps = psB.tile([P, CW], F32, tag="bps",
                                          name=f"qnps{c}_{h}_{m}")
                            up_chain(ps, 0, KQ, wqb_sb, h * P, P, b0, bw)
                            nc.scalar.activation(
                                q8_c[:, 0, h, 0, b0:b0 + bw],
                                ps[:, b0:b0 + bw], AF.Copy, scale=S_QN / S_B)
                            nc.vector.scalar_tensor_tensor(
                                q8_c[:, 1, h, 0, b0:b0 + bw],
                                ps[:, b0:b0 + bw], S_QN / S_B,
                                q8_c[:, 0, h, 0, b0:b0 + bw],
                                op0=MUL, op1=SUB)

                    # q-rope: stationary = latents (tokens), moving = w pe
                    for tt in range(NT):
                        ts = tt * P
                        pspe_t = psB.tile([P, CW], F32, tag="bps",
                                          name=f"pspe{c}_{tt}")
                        pspe = pspe_t[:, 0:HL * ROPE]
                        pc = HL * P
                        for i in range(KQ // 2):
                            nc.tensor.matmul(
                                pspe, qcn_c[:, 2 * i:2 * i + 2, 0, ts:ts + P],
                                wqb_sb[:, 2 * i:2 * i + 2, 1,
                                       pc:pc + HL * ROPE],
                                start=(i == 0), stop=False, perf_mode=DR)
                        for k in range(KQ):
                            nc.tensor.matmul(
                                pspe, qcn_c[:, k, :, ts:ts + P],
                                wqb_sb[:, k, :, pc:pc + HL * ROPE],
                                start=False, stop=(k == KQ - 1), perf_mode=DR)
                        t_glob = KT0 + tt
                        qr = scrB.tile([P, HL, ROPE], BF16, tag="qr")
                        qtmp = scrB.tile([P, HL, ROPE], BF16, tag="qtmp")
                        pspe_v = pspe_t[:, 0:HL * ROPE].rearrange(
                            "p (h r) -> p h r", r=ROPE)
                        cos_b = costok[:, t_glob].unsqueeze(1).broadcast_to(
                            [P, HL, ROPE])
                        nc.vector.tensor_mul(qr[:], pspe_v, cos_b)
                        sin_b1 = sintok[:, t_glob, 0:32].unsqueeze(
                            1).broadcast_to([P, HL, 32])
                        sin_b2 = sintok[:, t_glob, 32:ROPE].unsqueeze(
                            1).broadcast_to([P, HL, 32])
                        nc.vector.tensor_mul(qtmp[:, :, 0:32],
                                             pspe_v[:, :, 32:ROPE], sin_b1)
                        nc.vector.tensor_mul(qtmp[:, :, 32:ROPE],
                                             pspe_v[:, :, 0:32], sin_b2)
                        nc.vector.tensor_add(qr[:], qr[:], qtmp[:])
                        for h in range(HL):
                            tr = tr_tile()
                            nc.tensor.transpose(tr[0:ROPE, :], qr[:, h],
                                                ident_sb[:])
                            nc.scalar.activation(
                                q8_c[0:ROPE, 0, h, 1, ts:ts + P],
                                tr[0:ROPE, :], AF.Copy, scale=S_QPE)
                            nc.vector.scalar_tensor_tensor(
                                q8_c[0:ROPE, 1, h, 1, ts:ts + P],
                                tr[0:ROPE, :], S_QPE,
                                q8_c[0:ROPE, 0, h, 1, ts:ts + P],
                                op0=MUL, op1=SUB)

                    # ---- C: attention for this chunk's queries ----
                    attn8 = attnp.tile([P, HL, 2, CW], F8, tag="attn",
                                       name=f"attn{c}")


                    def dr3_scores(ss, h, kt, ps0, qs0, wd):
                        stat_hi = kn8[:, 0, h:HL + 1:HL - h,
                                      kt * P:(kt + 1) * P]
                        stat_lo = kn8[:, 1, h:HL + 1:HL - h,
                                      kt * P:(kt + 1) * P]
                        nc.tensor.matmul(
                            ss[:, ps0:ps0 + wd], stat_hi,
                            q8_c[:, 0, h, :, qs0:qs0 + wd],
                            start=True, stop=False, perf_mode=DR)
                        nc.tensor.matmul(
                            ss[:, ps0:ps0 + wd], stat_lo,
                            q8_c[:, 0, h, :, qs0:qs0 + wd],
                            start=False, stop=False, perf_mode=DR)
                        nc.tensor.matmul(
                            ss[:, ps0:ps0 + wd], stat_hi,
                            q8_c[:, 1, h, :, qs0:qs0 + wd],
                            start=False, stop=True, perf_mode=DR)

                    def emit_tail(h, pts):
                        """PV + denom + division for head h (interleaved
                        into the next head's score loop)."""
                        pv_ps = psC.tile([P, CW], F32, tag="cps",
                                         name=f"pv{c}_{h}")
                        for kt in range(nkt):
                            nc.tensor.matmul(
                                pv_ps[:, 0:W], v_sb[:, h, kt, :],
                                pts[kt], start=(kt == 0),
                                stop=(kt == nkt - 1))
                        dn = psB.tile([P, CW], F32, tag="bps",
                                      name=f"dn{c}_{h}")
                        for kt in range(nkt):
                            nc.tensor.matmul(
                                dn[0:1, 0:W], oratt_sb[:], pts[kt],
                                start=(kt == 0), stop=(kt == nkt - 1))
                        rrow = scrB.tile([1, CW], BF16, tag="rrow")
                        with nc.allow_low_precision(reason="softmax denom recip"):
                            nc.vector.reciprocal(rrow[0:1, 0:W], dn[0:1, 0:W])
                        rb = rbp.tile([P, CW], BF16, tag="rb")
                        nc.gpsimd.partition_broadcast(rb[:, 0:W],
                                                      rrow[0:1, 0:W])
                        prod = scrB.tile([P, CW], BF16, tag="pda")
                        nc.vector.tensor_mul(prod[:, 0:W], pv_ps[:, 0:W],
                                             rb[:, 0:W])
                        nc.scalar.activation(attn8[:, h, 0, 0:W],
                                             prod[:, 0:W], AF.Copy)
                        nc.vector.scalar_tensor_tensor(
                            attn8[:, h, 1, 0:W], prod[:, 0:W], 1.0,
                            attn8[:, h, 0, 0:W], op0=MUL, op1=SUB)

                    prev = None
                    for h in range(HL):
                        pts = []
                        if W == P:
                            # tail chunk: pack 4 key tiles per psum bank
                            for qd in range(nkt // 4):
                                ss = psC.tile([P, CW], F32, tag="cps")
                                for j in range(4):
                                    kt = qd * 4 + j
                                    dr3_scores(ss, h, kt, j * P, 0, P)
                                pt = ptp.tile([P, CW], BF16, tag="pt")
                                nc.scalar.activation(pt[:], ss[:], AF.Exp,
                                                     scale=EXP_SCALE)
                                for j in range(4):
                                    kt = qd * 4 + j
                                    if kt >= KT0:
                                        nc.vector.tensor_mul(
                                            pt[:, j * P:(j + 1) * P],
                                            pt[:, j * P:(j + 1) * P],
                                            masks_sb[:, 384:384 + P])
                                    pts.append(pt[:, j * P:(j + 1) * P])
                                if qd == 1 and prev is not None:
                                    emit_tail(*prev)
                                    prev = None
                        else:
                            for kt in range(nkt):
                                diag = (kt >= KT0)
                                off = (kt - KT0) * P if diag else 0
                                ss = psC.tile([P, CW], F32, tag="cps")
                                dr3_scores(ss, h, kt, 0, 0, W)
                                pt = ptp.tile([P, CW], BF16, tag="pt")
                                nc.scalar.activation(pt[:, 0:W], ss[:, 0:W],
                                                     AF.Exp, scale=EXP_SCALE)
                                if diag:
                                    nc.vector.tensor_mul(
                                        pt[:, 0:off + P],
                                        pt[:, 0:off + P],
                                        masks_sb[:, 384 - off:384 + P])
                                pts.append(pt[:, 0:W])
                                if kt == min(5, nkt - 1) and prev is not None:
                                    emit_tail(*prev)
                                    prev = None
                        if prev is not None:
                            emit_tail(*prev)
                        prev = (h, pts)
                    emit_tail(*prev)

                    # ---- D: wo projection for this chunk ----
                    for og in range(4):
                        ob = obp.tile([P, 4, CW], BF16, tag="ob")
                        for oi in range(4):
                            ot = og * 4 + oi
                            ps = psB.tile([P, CW], F32, tag="bps",
                                          name=f"dps{c}_{og}_{oi}")
                            oc = ot * P
                            for ws, wd in wslices(0, W):
                                psl = ps[:, ws:ws + wd]
                                for i in range(HL // 2):
                                    nc.tensor.matmul(
                                        psl,
                                        wo_sb[:, 2 * i:2 * i + 2, 1,
                                              oc:oc + P],
                                        attn8[:, 2 * i:2 * i + 2, 0,
                                              ws:ws + wd],
                                        start=(i == 0), stop=False,
                                        perf_mode=DR)
                                for k in range(HL):
                                    nc.tensor.matmul(
                                        psl, wo_sb[:, k, :, oc:oc + P],
                                        attn8[:, k, :, ws:ws + wd],
                                        start=False, stop=(k == HL - 1),
                                        perf_mode=DR)
                            nc.vector.tensor_scalar_mul(
                                ob[:, oi, 0:W], ps[:, 0:W],
                                1.0 / (S_ATT * S_WO))
                        nc.sync.dma_start(
                            outT[og * 4 * P:(og + 1) * 4 * P,
                                 c0:c0 + W].rearrange("(o p) t -> p o t", p=P),
                            ob[:, :, 0:W])

    nc.compile()
    return nc


_NC_CACHE = {}
_LAST_RES = None


def _get_nc(stages="ALL"):
    if "nc" not in _NC_CACHE:
        _NC_CACHE["nc"] = _build_nc()
    return _NC_CACHE["nc"]


def kernel(hidden_states, position_ids, wq_a, q_a_ln_w, wq_b, wkv_a, kv_a_ln_w,
           wkv_b, wo):
    hidden_states = np.asarray(hidden_states, dtype=np.float32)
    position_ids = np.asarray(position_ids)
    wq_a = np.asarray(wq_a, dtype=np.float32)
    wq_b = np.asarray(wq_b, dtype=np.float32)
    wkv_a = np.asarray(wkv_a, dtype=np.float32)
    wkv_b = np.asarray(wkv_b, dtype=np.float32)
    wo = np.asarray(wo, dtype=np.float32)
    # fold RMSNorm elementwise weights into the up-projections (exact)
    wq_b = wq_b * np.asarray(q_a_ln_w, dtype=np.float32)[None, :]
    wkv_b = wkv_b * np.asarray(kv_a_ln_w, dtype=np.float32)[None, :]
    assert hidden_states.shape == (B, S, HIDDEN)

    cos_t, sin_t = _yarn_cos_sin(S)

    # --- weight preprocessing (shared across cores in each batch group) ---
    wqb8_groups = []
    wkvb8_groups = []
    wo8_groups = []
    for g in range(4):
        heads = range(4 * g, 4 * g + 4)
        rows = []
        for h in heads:
            rows.append(np.arange(h * QHD, h * QHD + NOPE))
        pe_rows = []
        for h in heads:
            pe_rows.append(h * QHD + NOPE + _PERM64)
        rows = np.concatenate(rows + pe_rows)
        wqb8_groups.append(_stack_w(wq_b[rows].T, S_WB))

        rows = []
        for h in heads:
            rows.append(np.arange(h * (NOPE + VHD), h * (NOPE + VHD) + NOPE))
        for h in heads:
            rows.append(np.arange(h * (NOPE + VHD) + NOPE, (h + 1) * (NOPE + VHD)))
        rows = np.concatenate(rows)
        wkvb8_groups.append(_stack_w(wkv_b[rows].T, S_WB))

        cols = np.concatenate([np.arange(h * VHD, (h + 1) * VHD) for h in heads])
        wo8_groups.append(_stack_w(wo[:, cols].T, S_WO))

    wqa8 = _stack_w(wq_a.T, S_WA)
    wkva_perm = wkv_a.copy()
    wkva_perm[KV_LORA:] = wkv_a[KV_LORA + _PERM64]
    wkva8 = _stack_w(wkva_perm.T, S_WA)

    x_idx = np.arange(896)[None, :]
    p_idx = np.arange(P)[:, None]
    masks = _bf16((x_idx >= 384 + p_idx).astype(np.float32))

    # --- per-core inputs ---
    in_maps = []
    for c in range(N_CORES):
        beta, g = c // 4, c % 4
        pos = position_ids[beta].astype(np.int64)
        cos_g = cos_t[pos] / S_A  # stage-A k-rope tables fold A psum descale
        sin_g = sin_t[pos] / S_A
        cg = cos_t[pos] / S_B     # q-rope tables fold the B psum descale
        sg = sin_t[pos] / S_B
        sin_s = np.concatenate([-sg[:, :32], sg[:, 32:]], axis=1)

        own = np.concatenate(
            [np.arange(CHUNKS[0][1])] +
            [np.arange(t0 + g * (w // 4), t0 + (g + 1) * (w // 4))
             for t0, w in CHUNKS[1:1 + NGATH]] +
            [np.arange(CHUNKS[4][0], CHUNKS[4][0] + CHUNKS[4][1])])
        hT8v = _stack_a(hidden_states[beta].T[:, own], S_H)
        cosA = _bf16(cos_g[own].T)
        sg_own = sin_g[own]
        sinA = _bf16(np.concatenate([-sg_own[:, :32].T, sg_own[:, 32:].T], axis=0))

        in_maps.append({
            "hT8": hT8v,
            "wqa8": wqa8,
            "wkva8": wkva8,
            "wqb8": wqb8_groups[g],
            "wkvb8": wkvb8_groups[g],
            "wo8": wo8_groups[g],
            "cosA": cosA,
            "sinA": sinA,
            "cos_tm": _bf16(cg),
            "sin_tm_s": _bf16(sin_s),
            "masks": masks,
        })

    nc = _get_nc()
    global _LAST_RES
    res = run_bass_kernel_spmd(nc, in_maps, core_ids=list(range(N_CORES)))
    _LAST_RES = res

    out = np.zeros((B, S, HIDDEN), dtype=np.float32)
    for c in range(N_CORES):
        out[c // 4] += res.results[c]["outT"].astype(np.float32).T
    return out
